# revision 19
# baseline (speedup 1.0000x reference)
"""Trainium2 Bass kernel for the MoE-routed adapter problem.

Reference computation (M=2 routers, N=8 adapters, C=1024, D=256, B=32, S=512):
    per (m, b):  e = expert_index[m, b]
                 z = silu(x[b] @ down_w[m, e] + down_b[m, e])   # [S, D]
                 u = z @ up_w[m, e]                              # [S, C]
    out[m, b] = u                                                # [M, B, S, C]

Strategy: data-parallel over B across the 8 NeuronCores (4 batch elements per
core).  The expert gather is done host-side (numpy take_along_axis), weights
and activations are packed host-side into SBUF-layout contiguous blocks so
every device DMA is a single contiguous transfer.  Compute is bf16 matmuls
with fp32 PSUM accumulation (1 cycle/row on TensorE vs 4 for fp32); silu+bias
is fused into the ScalarEngine activation on the PSUM->SBUF path.

All matmuls are out[M,N] = lhsT[K,M].T @ rhs[K,N]:
  down: lhsT = down_w chunk [c:128, d:128], rhs = xT chunk [c:128, s:512]
        accumulated over 8 c-chunks -> zT [d:128, s:512] (transposed layout,
        exactly what the up matmul needs as its stationary operand)
  up:   lhsT = zT chunk [d:128, s:128], rhs = up_w chunk [d:128, c:512]
        accumulated over 2 d-chunks -> u [s:128, c:512] (natural layout)

DMA rings: inputs split between the Sync ring (x) and GpSimd SWDGE (weights)
so they stream in parallel; outputs go on the Scalar ring.  PSUM->SBUF output
casts are split 3:1 between VectorE and ScalarE to balance engine load.
"""

import sys

if "/opt/trn_rl_repo" not in sys.path:
    sys.path.insert(0, "/opt/trn_rl_repo")

import numpy as np
import ml_dtypes

BF16 = ml_dtypes.bfloat16

M, N, C, D = 2, 8, 1024, 256
B, S = 32, 512
NCORES = 8
BL = B // NCORES  # local batch per core
CK = C // 128     # 8 contraction chunks for the down matmul
DK = D // 128     # 2 d chunks
SK = S // 128     # 4 s chunks
NC2 = C // 512    # 2 output column chunks of 512

_compiled = None


def _build():
    from concourse import bacc, tile, mybir

    f32 = mybir.dt.float32
    bf16 = mybir.dt.bfloat16
    Silu = mybir.ActivationFunctionType.Silu

    nc = bacc.Bacc(
        "TRN2", target_bir_lowering=False, debug=False, num_devices=NCORES
    )

    # Per-core DRAM parameters, already packed host-side into SBUF layout:
    #   xt : [BL, 128, CK*S]    xt[b][p, ck*S + s]            = x[b, s, ck*128+p]
    #   dw : [M, BL, 128, 2048] dw[m,b][p, dk*1024 + ck*128 + j] = dw[ck*128+p, dk*128+j]
    #   uw : [M, BL, 128, 2048] uw[m,b][p, dk*1024 + c]         = uw[dk*128+p, c]
    #   db : [128, M*BL*DK]     db[p, (m*BL+b)*DK + dk]         = b_down[dk*128+p]
    #   out: [M, BL, 128, SK*C] out[m,b][p, sk*C + c]           = u[sk*128+p, c]
    xt_d = nc.dram_tensor("xt", [BL, 128, CK * S], bf16, kind="ExternalInput")
    dw_d = nc.dram_tensor("dw", [M, BL, 128, CK * D // 128 * 128], bf16,
                          kind="ExternalInput")
    uw_d = nc.dram_tensor("uw", [M, BL, 128, DK * C], bf16, kind="ExternalInput")
    db_d = nc.dram_tensor("db", [128, M * BL * DK], f32, kind="ExternalInput")
    out_d = nc.dram_tensor("out", [M, BL, 128, SK * C], bf16, kind="ExternalOutput")

    with tile.TileContext(nc) as tc:
        with (
            tc.tile_pool(name="const", bufs=1) as cpool,
            tc.tile_pool(name="warm", bufs=1) as warmpool,
            tc.tile_pool(name="xp", bufs=4) as xpool,
            tc.tile_pool(name="dwp", bufs=4) as dwpool,
            tc.tile_pool(name="uwp", bufs=4) as uwpool,
            tc.tile_pool(name="zp", bufs=3) as zpool,
            tc.tile_pool(name="up", bufs=3) as upool,
            tc.tile_pool(name="zpsum", bufs=2, space="PSUM") as zpsum,
            tc.tile_pool(name="upsum", bufs=6, space="PSUM") as upsum,
        ):
            # PE warm-up: dummy matmuls on memset scratch while the first
            # input DMAs are in flight, so HAM un-throttles the PE clock
            # before the real matmuls start (and the PE never sits cold).
            wsrc = warmpool.tile([128, 512], bf16, name="wsrc")
            nc.gpsimd.memset(wsrc[:], 0.0)
            pwarm = zpsum.tile([128, S], f32, name="pz")
            for _ in range(14):
                nc.tensor.matmul(
                    pwarm[:], wsrc[:, :128], wsrc[:], start=True, stop=True
                )

            dbt = cpool.tile([128, M * BL * DK], f32)
            nc.gpsimd.dma_start(out=dbt[:], in_=db_d.ap())

            for b in range(BL):
                # Steady state: x halves on the Sync HWDGE ring, weights on
                # GpSimd SWDGE — parallel DMA queue rows.  b==0 is
                # latency-critical (nothing to overlap with), so its inputs
                # are spread across all three rings in PE consumption order:
                # m0 dk0 weights + x quarters first on the two fast HWDGE
                # rings, later-needed tensors behind them.
                if b == 0:
                    # b==0 fill is latency-critical: m0 dk0 weights lead the
                    # Sync ring, x quarters alternate across both HWDGE
                    # rings so the down accumulation unblocks chunk by chunk
                    d0a = dwpool.tile([128, 1024], bf16, name="dwt0a")
                    nc.sync.dma_start(out=d0a[:], in_=dw_d.ap()[0, b][:, :1024])
                    xqs = []
                    for q in range(4):
                        xq = xpool.tile([128, 2 * S], bf16, name="xt4")
                        eng = nc.sync if q % 2 == 0 else nc.scalar
                        eng.dma_start(
                            out=xq[:], in_=xt_d.ap()[b][:, q * 1024 : (q + 1) * 1024]
                        )
                        xqs.append(xq)
                    d0b = dwpool.tile([128, 1024], bf16, name="dwt0b")
                    nc.sync.dma_start(out=d0b[:], in_=dw_d.ap()[0, b][:, 1024:])
                    d1 = dwpool.tile([128, 2048], bf16, name="dwt")
                    nc.gpsimd.dma_start(out=d1[:], in_=dw_d.ap()[1, b])

                    xslice = lambda ck, _x=xqs: _x[ck // 2][
                        :, (ck % 2) * S : (ck % 2 + 1) * S
                    ]
                    dwslice = [
                        lambda dk, ck, _a=d0a, _b=d0b: (_a if dk == 0 else _b)[
                            :, ck * 128 : (ck + 1) * 128
                        ],
                        lambda dk, ck, _t=d1: _t[
                            :, dk * 1024 + ck * 128 : dk * 1024 + (ck + 1) * 128
                        ],
                    ]
                    ck_order = list(range(CK))

                    uwts = []
                    for m in range(M):
                        uwt = uwpool.tile([128, 2048], bf16, name="uwt")
                        nc.gpsimd.dma_start(out=uwt[:], in_=uw_d.ap()[m, b])
                        uwts.append(uwt)
                else:
                    xts = []
                    for h in range(2):
                        xh = xpool.tile([128, 4 * S], bf16, name="xt")
                        nc.sync.dma_start(
                            out=xh[:], in_=xt_d.ap()[b][:, h * 2048 : (h + 1) * 2048]
                        )
                        xts.append(xh)
                    xslice = lambda ck, _x=xts: _x[ck // 4][
                        :, (ck % 4) * S : (ck % 4 + 1) * S
                    ]
                    dwts = []
                    for m in range(M):
                        dwt = dwpool.tile([128, 2048], bf16, name="dwt")
                        nc.gpsimd.dma_start(out=dwt[:], in_=dw_d.ap()[m, b])
                        dwts.append(dwt)
                    dwslice = [
                        (
                            lambda dk, ck, _t=t: _t[
                                :, dk * 1024 + ck * 128 : dk * 1024 + (ck + 1) * 128
                            ]
                        )
                        for t in dwts
                    ]
                    ck_order = list(range(CK))

                    uwts = []
                    for m in range(M):
                        uwt = uwpool.tile([128, 2048], bf16, name="uwt")
                        nc.gpsimd.dma_start(out=uwt[:], in_=uw_d.ap()[m, b])
                        uwts.append(uwt)

                # down projection + silu for both routers first so the up
                # matmuls of router m overlap the activation of router m+1
                zts = []
                for m in range(M):
                    zt = zpool.tile([128, DK, S], bf16, name="zt")
                    for dk in range(DK):
                        pz = zpsum.tile([128, S], f32, name="pz")
                        for ci, ck in enumerate(ck_order):
                            nc.tensor.matmul(
                                pz[:],
                                dwslice[m](dk, ck),
                                xslice(ck),
                                start=(ci == 0),
                                stop=(ci == CK - 1),
                            )
                        col = (m * BL + b) * DK + dk
                        nc.scalar.activation(
                            zt[:, dk, :], pz[:], Silu, bias=dbt[:, col : col + 1]
                        )
                    zts.append(zt)

                for m in range(M):
                    ut = upool.tile([128, SK * C], bf16, name="ut")
                    for sk in range(SK):
                        pus = [
                            upsum.tile([128, 512], f32, name="pu") for _ in range(NC2)
                        ]
                        for dk in range(DK):
                            for ncol in range(NC2):
                                nc.tensor.matmul(
                                    pus[ncol][:],
                                    zts[m][:, dk, sk * 128 : (sk + 1) * 128],
                                    uwts[m][
                                        :,
                                        dk * 1024 + ncol * 512 : dk * 1024
                                        + (ncol + 1) * 512,
                                    ],
                                    start=(dk == 0),
                                    stop=(dk == DK - 1),
                                )
                        # drain PSUM->SBUF(bf16) split across both engines so
                        # neither becomes the up-phase bottleneck
                        for ncol in range(NC2):
                            dst = ut[:, sk * C + ncol * 512 : sk * C + (ncol + 1) * 512]
                            if ncol == 0:
                                nc.vector.tensor_copy(dst, pus[ncol][:])
                            else:
                                nc.scalar.copy(dst, pus[ncol][:])
                        last_tile = b == BL - 1 and m == M - 1
                        if last_tile:
                            if sk == SK - 1:
                                # very last quarter: halve across both rings
                                nc.sync.dma_start(
                                    out=out_d.ap()[m, b][:, sk * C : sk * C + 512],
                                    in_=ut[:, sk * C : sk * C + 512],
                                )
                                nc.scalar.dma_start(
                                    out=out_d.ap()[m, b][:, sk * C + 512 : (sk + 1) * C],
                                    in_=ut[:, sk * C + 512 : (sk + 1) * C],
                                )
                            else:
                                eng = nc.sync if sk % 2 == 0 else nc.scalar
                                eng.dma_start(
                                    out=out_d.ap()[m, b][:, sk * C : (sk + 1) * C],
                                    in_=ut[:, sk * C : (sk + 1) * C],
                                )
                        else:
                            if sk == 1:
                                nc.scalar.dma_start(
                                    out=out_d.ap()[m, b][:, : 2 * C],
                                    in_=ut[:, : 2 * C],
                                )
                            if sk == SK - 1:
                                nc.sync.dma_start(
                                    out=out_d.ap()[m, b][:, 2 * C :],
                                    in_=ut[:, 2 * C :],
                                )

    nc.compile()
    return nc


def _get_compiled():
    global _compiled
    if _compiled is None:
        _compiled = _build()
    return _compiled


def _pack_inputs(x, expert_index, down_w, down_b, up_w):
    idx = expert_index.astype(np.int64)
    dwg = np.take_along_axis(down_w, idx[:, :, None, None], axis=1)  # [M,B,C,D]
    dbg = np.take_along_axis(down_b, idx[:, :, None], axis=1)        # [M,B,D]
    uwg = np.take_along_axis(up_w, idx[:, :, None, None], axis=1)    # [M,B,D,C]

    # x -> [B, 128, CK*S]: xt[b, p, ck*S+s] = x[b, s, ck*128+p]
    xt = (
        x.transpose(0, 2, 1)                     # [B, C, S]
        .reshape(B, CK, 128, S)
        .transpose(0, 2, 1, 3)                   # [B, 128, CK, S]
        .reshape(B, 128, CK * S)
        .astype(BF16)
    )
    # down_w -> [M, B, 128, 2048]: [p, dk*1024 + ck*128 + j] = dw[ck*128+p, dk*128+j]
    dwp = (
        dwg.reshape(M, B, CK, 128, DK, 128)
        .transpose(0, 1, 3, 4, 2, 5)             # [M, B, 128, DK, CK, 128]
        .reshape(M, B, 128, 2048)
        .astype(BF16)
    )
    # up_w -> [M, B, 128, 2048]: [p, dk*1024 + c] = uw[dk*128+p, c]
    uwp = (
        uwg.reshape(M, B, DK, 128, C)
        .transpose(0, 1, 3, 2, 4)                # [M, B, 128, DK, C]
        .reshape(M, B, 128, 2048)
        .astype(BF16)
    )

    in_maps = []
    for c in range(NCORES):
        bs = slice(c * BL, (c + 1) * BL)
        dbc = (
            dbg[:, bs]
            .reshape(M, BL, DK, 128)
            .transpose(3, 0, 1, 2)               # [128, M, BL, DK]
            .reshape(128, M * BL * DK)
            .astype(np.float32)
        )
        in_maps.append(
            {
                "xt": np.ascontiguousarray(xt[bs]),
                "dw": np.ascontiguousarray(dwp[:, bs]),
                "uw": np.ascontiguousarray(uwp[:, bs]),
                "db": np.ascontiguousarray(dbc),
            }
        )
    return in_maps


def kernel(x, expert_index, down_w, down_b, up_w, _run_kwargs=None):
    nc = _get_compiled()
    in_maps = _pack_inputs(
        np.asarray(x, dtype=np.float32),
        np.asarray(expert_index),
        np.asarray(down_w, dtype=np.float32),
        np.asarray(down_b, dtype=np.float32),
        np.asarray(up_w, dtype=np.float32),
    )

    from concourse.bass_utils import run_bass_kernel_spmd

    res = run_bass_kernel_spmd(
        nc, in_maps, core_ids=list(range(NCORES)), **(_run_kwargs or {})
    )

    out = np.empty((M, B, S, C), dtype=np.float32)
    for c in range(NCORES):
        buf = np.asarray(res.results[c]["out"])        # [M, BL, 128, SK*C] bf16
        out[:, c * BL : (c + 1) * BL] = (
            buf.astype(np.float32)
            .reshape(M, BL, 128, SK, C)
            .transpose(0, 1, 3, 2, 4)                  # [M, BL, SK, 128, C]
            .reshape(M, BL, S, C)
        )
    globals()["_last_results"] = res
    return out
